# revision 16
# baseline (speedup 1.0000x reference)
"""Multi-step LIF neuron (T=4) on 8 Trainium2 NeuronCores via Bass/Tile.

Reference recurrence (per element, v0 = 0, tau = 2, v_th = 1, hard reset to 0):
    v_c  = v + (x - v) * 0.5        # exact reference op order (bit-exact)
    s    = (v_c >= 1.0)             # spike (forward value of the STE)
    v'   = 0 if s else v_c
Output is s as float32 (0.0 / 1.0), shape [4, 128, 262144].

Sharding: pure data parallel over batch. B=128 = 8 cores x 16 rows; each core
computes x_shard [4, 128, 32768] -> spike shard of the same shape. The T
recurrence is carried per element in SBUF; no cross-core communication.

Implementation notes:
  - One fused custom DVE op per (step, output): LIF_SPIKE computes
    (v + (x-v)*0.5 >= 1) in a single 1-elem/cycle pass; LIF_VNEXT computes
    the reset potential. 7 DVE passes per element total (vs ~20 with
    standard ops).
  - Spikes are written as uint8 on device (exact 0/1) and widened to f32 on
    the host, cutting store-side HBM traffic 4x. The kernel is HBM-bound:
    ~64 MiB in + 16 MiB out per core at ~358 GB/s.
"""

import numpy as np

import concourse.bass as bass
import concourse.mybir as mybir
import concourse.tile as tile
from concourse import bacc
import concourse.dve_ops as dve_ops
from concourse.dve_spec import Spec, Src0, Src1, C0, C1, Zero, select, lower, _has_src1
from concourse.dve_uop import DveOpSpec
from concourse.bass_utils import run_bass_kernel_spmd

F32 = mybir.dt.float32
U8 = mybir.dt.uint8

T = 4
B = 128
N = 262144
N_CORES = 8
ROWS_PER_CORE = B // N_CORES              # 16
FREE = ROWS_PER_CORE * N // 128           # 32768 free elems per partition
P = 128
TILE_F = 2048                             # free-dim tile: 1 MiB f32 per DMA

_cache = {}


# ------------------------------------------------------------ custom DVE ops
def _register(name, spec, perf_en=False):
    for op in dve_ops.OPS:
        if op.name == name:
            return op
    opcode = dve_ops._CUSTOM_DVE_ROW_BASE + len(dve_ops.OPS)
    assert opcode < 0x20, "custom DVE opcode rows exhausted"
    dve_ops._SUB_OPCODE_FOR_NAME[name] = opcode
    shas = {}
    for ver in ("v3", "v4"):
        try:
            u = lower(spec, ver=ver)
            s = DveOpSpec(name=name, opcode=opcode, uops=u, rd1_en=_has_src1(spec))
            shas[ver] = s.sha(ver)
        except Exception:
            pass
    op = dve_ops.DveOp(name, spec, subdim=False, uops_sha=shas,
                       perf_en={"v3": perf_en, "v4": perf_en} if perf_en else {})
    dve_ops.OPS.append(op)
    dve_ops.CUSTOM_DVE_SPECS[name] = spec
    return op


# s0 = tau reciprocal (0.5), s1 = threshold (1.0)
_vc = Src1 + (Src0 - Src1) * C0            # in0 = x, in1 = v
LIF_SPIKE = _register("LIF_SPIKE", Spec(body=(_vc >= C1)))
LIF_VNEXT = _register("LIF_VNEXT", Spec(body=select(_vc >= C1, Zero, _vc)))
_vc0 = Src0 * C0                           # first step: v = 0
LIF0_VNEXT = _register("LIF0_VNEXT", Spec(body=select(_vc0 >= C1, Zero, _vc0)),
                       perf_en=True)


# ------------------------------------------------------------------ bass build
NJ = FREE // TILE_F                       # 16 j-tiles per core


def _build_nc(rep: int = 1):
    # DRAM layout (host-prepared): x[p, j, t, f], s[p, j, t, f] — one 4 MiB
    # load and one 1 MiB store per j, with 32 KiB contiguous partition lines.
    nc = bacc.Bacc("TRN2", target_bir_lowering=False)
    x_d = nc.declare_dram_parameter("x", [P, NJ, T * TILE_F], F32, isOutput=False)
    s_d = nc.declare_dram_parameter("s", [P, NJ, T * TILE_F], U8, isOutput=True)
    scratch = [
        nc.dram_tensor(f"s_scratch{r}", [P, NJ, T * TILE_F], U8)
        for r in range(rep - 1)
    ]

    with tile.TileContext(nc) as tc:
        with tc.tile_pool(name="io", bufs=3) as io, tc.tile_pool(name="work", bufs=1) as work:
            for r in range(rep):
                out_d = s_d if r == 0 else scratch[r - 1]
                for j in range(NJ):
                    xa = io.tile([P, T * TILE_F], F32, tag="x")
                    nc.sync.dma_start(out=xa[:], in_=x_d[:, j, :])
                    sa = io.tile([P, T * TILE_F], U8, tag="s")
                    xt = [xa[:, bass.ts(t, TILE_F)] for t in range(T)]
                    st = [sa[:, bass.ts(t, TILE_F)] for t in range(T)]
                    va = work.tile([P, TILE_F], F32, tag="va")
                    vb = work.tile([P, TILE_F], F32, tag="vb")
                    # t = 0:  v_c = x*0.5 ;  spike = (x*0.5 >= 1) == (x >= 2)
                    nc.vector.tensor_scalar(st[0], xt[0], 2.0, None,
                                            mybir.AluOpType.is_ge)
                    nc.vector._custom_dve(LIF0_VNEXT, out=va[:], in0=xt[0],
                                          s0=0.5, s1=1.0)
                    vcur = va
                    for t in range(1, T):
                        nc.vector._custom_dve(
                            LIF_SPIKE, out=st[t], in0=xt[t], in1=vcur[:],
                            s0=0.5, s1=1.0)
                        if t < T - 1:
                            vnew = vb if vcur is va else va
                            nc.vector._custom_dve(
                                LIF_VNEXT, out=vnew[:], in0=xt[t], in1=vcur[:],
                                s0=0.5, s1=1.0)
                            vcur = vnew
                    nc.sync.dma_start(out=out_d[:, j, :], in_=sa[:])

    nc.compile()
    return nc


def _get_nc(rep: int = 1):
    key = f"nc{rep}"
    if key not in _cache:
        _cache[key] = _build_nc(rep)
    return _cache[key]


def _shard(x_seq: np.ndarray) -> list[dict[str, np.ndarray]]:
    # logical per-core view: [T, P, FREE] with (P, FREE) = flattened 16 B-rows;
    # DRAM layout for the kernel: [P, NJ, T, TILE_F]
    in_maps = []
    for c in range(N_CORES):
        xs = x_seq[:, c * ROWS_PER_CORE:(c + 1) * ROWS_PER_CORE, :].reshape(
            T, P, NJ, TILE_F)
        xs = np.ascontiguousarray(xs.transpose(1, 2, 0, 3)).reshape(
            P, NJ, T * TILE_F)
        in_maps.append({"x": xs})
    return in_maps


def _unshard(results: list[dict[str, np.ndarray]]) -> np.ndarray:
    parts = []
    for r in results:
        s = r["s"].reshape(P, NJ, T, TILE_F).transpose(2, 0, 1, 3)
        parts.append(s.reshape(T, ROWS_PER_CORE, N))
    return np.concatenate(parts, axis=1).astype(np.float32)


def kernel(x_seq: np.ndarray) -> np.ndarray:
    x_seq = np.asarray(x_seq, dtype=np.float32)
    assert x_seq.shape == (T, B, N), x_seq.shape
    nc = _get_nc()
    res = run_bass_kernel_spmd(nc, _shard(x_seq), core_ids=list(range(N_CORES)))
    return _unshard(res.results)


# ---------------------------------------------------------------- benchmarking
def _make_exec(nc):
    """Build the sharded jitted executable once (mirrors run_bass_via_pjrt)."""
    import jax
    from jax.sharding import Mesh, PartitionSpec
    from jax.experimental.shard_map import shard_map
    from concourse import bass2jax

    bass2jax.install_neuronx_cc_hook()

    partition_name = nc.partition_id_tensor.name if nc.partition_id_tensor else None
    in_names, out_names, out_avals, zero_outs = [], [], [], []
    for alloc in nc.m.functions[0].allocations:
        if not isinstance(alloc, mybir.MemoryLocationSet):
            continue
        name = alloc.memorylocations[0].name
        if alloc.kind == "ExternalInput":
            if name != partition_name:
                in_names.append(name)
        elif alloc.kind == "ExternalOutput":
            shape = tuple(alloc.tensor_shape)
            dtype = mybir.dt.np(alloc.dtype)
            out_names.append(name)
            out_avals.append(jax.core.ShapedArray(shape, dtype))
            zero_outs.append(np.zeros(shape, dtype))
    n_params = len(in_names)
    n_outs = len(out_avals)
    all_in_names = in_names + out_names
    if partition_name is not None:
        all_in_names.append(partition_name)
    donate = tuple(range(n_params, n_params + n_outs))

    def _body(*args):
        operands = list(args)
        if partition_name is not None:
            operands.append(bass2jax.partition_id_tensor())
        outs = bass2jax._bass_exec_p.bind(
            *operands,
            out_avals=tuple(out_avals),
            in_names=tuple(all_in_names),
            out_names=tuple(out_names),
            lowering_input_output_aliases=(),
            sim_require_finite=True,
            sim_require_nnan=True,
            nc=nc,
        )
        return tuple(outs)

    devices = jax.devices()[:N_CORES]
    mesh = Mesh(np.asarray(devices), ("core",))
    in_specs = (PartitionSpec("core"),) * (n_params + n_outs)
    out_specs = (PartitionSpec("core"),) * n_outs
    f = jax.jit(
        shard_map(_body, mesh=mesh, in_specs=in_specs, out_specs=out_specs,
                  check_rep=False),
        donate_argnums=donate, keep_unused=True,
    )
    return f, mesh, in_names, out_names, zero_outs


def _time_rep(x_seq, rep, repeats):
    import time
    import jax
    from jax.sharding import NamedSharding, PartitionSpec

    nc = _get_nc(rep)
    f, mesh, in_names, out_names, zero_outs = _make_exec(nc)

    in_maps = _shard(x_seq)
    concat_in = [
        np.concatenate([m[name] for m in in_maps], axis=0) for name in in_names
    ]
    sh = NamedSharding(mesh, PartitionSpec("core"))
    xc = [jax.device_put(a, sh) for a in concat_in]
    zc = [
        jax.device_put(np.zeros((N_CORES * z.shape[0], *z.shape[1:]), z.dtype), sh)
        for z in zero_outs
    ]
    outs = f(*xc, *zc)  # warm-up (compiles)
    jax.block_until_ready(outs)
    times = []
    for _ in range(repeats):
        t0 = time.perf_counter()
        outs = f(*xc, *outs)
        jax.block_until_ready(outs)
        times.append(time.perf_counter() - t0)
    times.sort()
    return times


def bench(x_seq: np.ndarray, repeats: int = 10, rep: int = 5):
    """Estimate per-execution device time: marginal cost of extra in-kernel
    repetitions of the full pipeline (cancels RPC/dispatch overhead)."""
    x_seq = np.asarray(x_seq, dtype=np.float32)
    t1 = _time_rep(x_seq, 1, repeats)
    tk = _time_rep(x_seq, rep, repeats)
    print(f"rep=1 times: {[f'{t:.6f}' for t in t1]}")
    print(f"rep={rep} times: {[f'{t:.6f}' for t in tk]}")
    marginal = (tk[0] - t1[0]) / (rep - 1)
    print(f"rep=1 min: {t1[0]*1e3:.3f} ms; rep={rep} min: {tk[0]*1e3:.3f} ms; "
          f"marginal per exec: {marginal*1e3:.3f} ms")
    return marginal * 1e9


# revision 19
# speedup vs baseline: 57082.8629x; 57082.8629x over previous
"""Multi-step LIF neuron (T=4) on 8 Trainium2 NeuronCores via Bass/Tile.

Reference recurrence (per element, v0 = 0, tau = 2, v_th = 1, hard reset to 0):
    v_c  = v + (x - v) * 0.5        # exact reference op order (bit-exact)
    s    = (v_c >= 1.0)             # spike (forward value of the STE)
    v'   = 0 if s else v_c
Output is s as float32 (0.0 / 1.0), shape [4, 128, 262144].

Sharding: pure data parallel over batch. B=128 = 8 cores x 16 rows; each core
computes x_shard [4, 128, 32768] -> spike shard of the same shape. The T
recurrence is carried per element in SBUF; no cross-core communication.

Implementation notes:
  - One fused custom DVE op per (step, output): LIF_SPIKE computes
    (v + (x-v)*0.5 >= 1) in a single 1-elem/cycle pass; LIF_VNEXT computes
    the reset potential. 7 DVE passes per element total (vs ~20 with
    standard ops).
  - Spikes are written as uint8 on device (exact 0/1) and widened to f32 on
    the host, cutting store-side HBM traffic 4x. The kernel is HBM-bound:
    ~64 MiB in + 16 MiB out per core at ~358 GB/s.
"""

import numpy as np

import concourse.bass as bass
import concourse.mybir as mybir
import concourse.tile as tile
from concourse import bacc
import concourse.dve_ops as dve_ops
from concourse.dve_spec import Spec, Src0, Src1, C0, C1, Zero, select, lower, _has_src1
from concourse.dve_uop import DveOpSpec
from concourse.bass_utils import run_bass_kernel_spmd

F32 = mybir.dt.float32
U8 = mybir.dt.uint8

T = 4
B = 128
N = 262144
N_CORES = 8
ROWS_PER_CORE = B // N_CORES              # 16
FREE = ROWS_PER_CORE * N // 128           # 32768 free elems per partition
P = 128
TILE_F = 2048                             # free-dim tile: 1 MiB f32 per DMA

_cache = {}


# ------------------------------------------------------------ custom DVE ops
def _register(name, spec, perf_en=False):
    for op in dve_ops.OPS:
        if op.name == name:
            return op
    opcode = dve_ops._CUSTOM_DVE_ROW_BASE + len(dve_ops.OPS)
    assert opcode < 0x20, "custom DVE opcode rows exhausted"
    dve_ops._SUB_OPCODE_FOR_NAME[name] = opcode
    shas = {}
    for ver in ("v3", "v4"):
        try:
            u = lower(spec, ver=ver)
            s = DveOpSpec(name=name, opcode=opcode, uops=u, rd1_en=_has_src1(spec))
            shas[ver] = s.sha(ver)
        except Exception:
            pass
    op = dve_ops.DveOp(name, spec, subdim=False, uops_sha=shas,
                       perf_en={"v3": perf_en, "v4": perf_en} if perf_en else {})
    dve_ops.OPS.append(op)
    dve_ops.CUSTOM_DVE_SPECS[name] = spec
    return op


# s0 = tau reciprocal (0.5), s1 = threshold (1.0)
_vc = Src1 + (Src0 - Src1) * C0            # in0 = x, in1 = v
LIF_SPIKE = _register("LIF_SPIKE", Spec(body=(_vc >= C1)))
LIF_VNEXT = _register("LIF_VNEXT", Spec(body=select(_vc >= C1, Zero, _vc)))
_vc0 = Src0 * C0                           # first step: v = 0
LIF0_VNEXT = _register("LIF0_VNEXT", Spec(body=select(_vc0 >= C1, Zero, _vc0)),
                       perf_en=True)


# ------------------------------------------------------------------ bass build
NJ = FREE // TILE_F                       # 16 j-tiles per core


def _build_nc(rep: int = 1):
    nc = bacc.Bacc("TRN2", target_bir_lowering=False)
    x_d = nc.declare_dram_parameter("x", [T, P, FREE], F32, isOutput=False)
    s_d = nc.declare_dram_parameter("s", [T, P, FREE], U8, isOutput=True)
    scratch = [
        nc.dram_tensor(f"s_scratch{r}", [T, P, FREE], U8) for r in range(rep - 1)
    ]

    with tile.TileContext(nc) as tc:
        with tc.tile_pool(name="io", bufs=3) as io, tc.tile_pool(name="work", bufs=1) as work:
            for r in range(rep):
                out_d = s_d if r == 0 else scratch[r - 1]
                for j in range(NJ):
                    js = bass.ts(j, TILE_F)
                    xt = []
                    for t in range(T):
                        xtile = io.tile([P, TILE_F], F32, tag=f"x{t}")
                        nc.sync.dma_start(out=xtile[:], in_=x_d[t, :, js])
                        xt.append(xtile)
                    st = []
                    for t in range(T):
                        stile = io.tile([P, TILE_F], U8, tag=f"s{t}")
                        st.append(stile)
                    va = work.tile([P, TILE_F], F32, tag="va")
                    vb = work.tile([P, TILE_F], F32, tag="vb")
                    # t = 0:  v_c = x*0.5 ;  spike = (x*0.5 >= 1) == (x >= 2)
                    nc.vector.tensor_scalar(st[0][:], xt[0][:], 2.0, None,
                                            mybir.AluOpType.is_ge)
                    nc.vector._custom_dve(LIF0_VNEXT, out=va[:], in0=xt[0][:],
                                          s0=0.5, s1=1.0)
                    vcur = va
                    for t in range(1, T):
                        nc.vector._custom_dve(
                            LIF_SPIKE, out=st[t][:], in0=xt[t][:], in1=vcur[:],
                            s0=0.5, s1=1.0)
                        if t < T - 1:
                            vnew = vb if vcur is va else va
                            nc.vector._custom_dve(
                                LIF_VNEXT, out=vnew[:], in0=xt[t][:], in1=vcur[:],
                                s0=0.5, s1=1.0)
                            vcur = vnew
                    for t in range(T):
                        nc.sync.dma_start(out=out_d[t, :, js], in_=st[t][:])

    nc.compile()
    return nc


def _get_nc(rep: int = 1):
    key = f"nc{rep}"
    if key not in _cache:
        _cache[key] = _build_nc(rep)
    return _cache[key]


def _shard(x_seq: np.ndarray) -> list[dict[str, np.ndarray]]:
    in_maps = []
    for c in range(N_CORES):
        xs = np.ascontiguousarray(
            x_seq[:, c * ROWS_PER_CORE:(c + 1) * ROWS_PER_CORE, :]
        ).reshape(T, P, FREE)
        in_maps.append({"x": xs})
    return in_maps


def _unshard(results: list[dict[str, np.ndarray]]) -> np.ndarray:
    parts = [r["s"].reshape(T, ROWS_PER_CORE, N) for r in results]
    return np.concatenate(parts, axis=1).astype(np.float32)


def kernel(x_seq: np.ndarray) -> np.ndarray:
    x_seq = np.asarray(x_seq, dtype=np.float32)
    assert x_seq.shape == (T, B, N), x_seq.shape
    nc = _get_nc()
    res = run_bass_kernel_spmd(nc, _shard(x_seq), core_ids=list(range(N_CORES)))
    return _unshard(res.results)


# ---------------------------------------------------------------- benchmarking
def _make_exec(nc):
    """Build the sharded jitted executable once (mirrors run_bass_via_pjrt)."""
    import jax
    from jax.sharding import Mesh, PartitionSpec
    from jax.experimental.shard_map import shard_map
    from concourse import bass2jax

    bass2jax.install_neuronx_cc_hook()

    partition_name = nc.partition_id_tensor.name if nc.partition_id_tensor else None
    in_names, out_names, out_avals, zero_outs = [], [], [], []
    for alloc in nc.m.functions[0].allocations:
        if not isinstance(alloc, mybir.MemoryLocationSet):
            continue
        name = alloc.memorylocations[0].name
        if alloc.kind == "ExternalInput":
            if name != partition_name:
                in_names.append(name)
        elif alloc.kind == "ExternalOutput":
            shape = tuple(alloc.tensor_shape)
            dtype = mybir.dt.np(alloc.dtype)
            out_names.append(name)
            out_avals.append(jax.core.ShapedArray(shape, dtype))
            zero_outs.append(np.zeros(shape, dtype))
    n_params = len(in_names)
    n_outs = len(out_avals)
    all_in_names = in_names + out_names
    if partition_name is not None:
        all_in_names.append(partition_name)
    donate = tuple(range(n_params, n_params + n_outs))

    def _body(*args):
        operands = list(args)
        if partition_name is not None:
            operands.append(bass2jax.partition_id_tensor())
        outs = bass2jax._bass_exec_p.bind(
            *operands,
            out_avals=tuple(out_avals),
            in_names=tuple(all_in_names),
            out_names=tuple(out_names),
            lowering_input_output_aliases=(),
            sim_require_finite=True,
            sim_require_nnan=True,
            nc=nc,
        )
        return tuple(outs)

    devices = jax.devices()[:N_CORES]
    mesh = Mesh(np.asarray(devices), ("core",))
    in_specs = (PartitionSpec("core"),) * (n_params + n_outs)
    out_specs = (PartitionSpec("core"),) * n_outs
    f = jax.jit(
        shard_map(_body, mesh=mesh, in_specs=in_specs, out_specs=out_specs,
                  check_rep=False),
        donate_argnums=donate, keep_unused=True,
    )
    return f, mesh, in_names, out_names, zero_outs


def _time_rep(x_seq, rep, repeats):
    import time
    import jax
    from jax.sharding import NamedSharding, PartitionSpec

    nc = _get_nc(rep)
    f, mesh, in_names, out_names, zero_outs = _make_exec(nc)

    in_maps = _shard(x_seq)
    concat_in = [
        np.concatenate([m[name] for m in in_maps], axis=0) for name in in_names
    ]
    sh = NamedSharding(mesh, PartitionSpec("core"))
    xc = [jax.device_put(a, sh) for a in concat_in]
    zc = [
        jax.device_put(np.zeros((N_CORES * z.shape[0], *z.shape[1:]), z.dtype), sh)
        for z in zero_outs
    ]
    outs = f(*xc, *zc)  # warm-up (compiles)
    jax.block_until_ready(outs)
    times = []
    for _ in range(repeats):
        t0 = time.perf_counter()
        outs = f(*xc, *outs)
        jax.block_until_ready(outs)
        times.append(time.perf_counter() - t0)
    times.sort()
    return times


def bench(x_seq: np.ndarray, repeats: int = 10, rep: int = 5):
    """Estimate per-execution device time: marginal cost of extra in-kernel
    repetitions of the full pipeline (cancels RPC/dispatch overhead)."""
    x_seq = np.asarray(x_seq, dtype=np.float32)
    t1 = _time_rep(x_seq, 1, repeats)
    tk = _time_rep(x_seq, rep, repeats)
    print(f"rep=1 times: {[f'{t:.6f}' for t in t1]}")
    print(f"rep={rep} times: {[f'{t:.6f}' for t in tk]}")
    marginal = (tk[0] - t1[0]) / (rep - 1)
    print(f"rep=1 min: {t1[0]*1e3:.3f} ms; rep={rep} min: {tk[0]*1e3:.3f} ms; "
          f"marginal per exec: {marginal*1e3:.3f} ms")
    return marginal * 1e9
